# revision 18
# baseline (speedup 1.0000x reference)
"""MoE MLP (9 experts, top-2 routing) on 8 TRN2 NeuronCores.

Strategy: expert-parallel. The router (tiny) runs on host CPU with the exact
jax ops of the reference so top-2 selection matches bitwise. Tokens are
gathered per expert on host; the largest expert is split across all 8 cores
(slot B), each core additionally owns one of the remaining 8 experts
(slot A). Every core runs the same SPMD Bass program (shapes baked from the
actual routing at call time): gate/up matmuls (bf16, fp32 PSUM), silu*up,
down matmul, all with features on partitions and tokens on the free dim so
no transposes are needed. Host applies combine weights and scatter-adds.

The matmul stream is PE-streaming-bound (~220ns/token column), so the
remaining edges are optimized: warmup matmuls on a zeroed tile keep the PE
busy (and the HAM clock-gate warming) through the startup DMA window, the
first-needed weight loads carry raised DMA priority, and the output is
returned as bf16 to halve the output drain.
"""

import os

# The tunneled NeuronCores can be left wedged (NRT_EXEC_UNIT_UNRECOVERABLE)
# by a previous process; resetting cores at NRT init makes runs reliable.
os.environ.setdefault("NEURON_RT_RESET_CORES", "1")

import numpy as np
import ml_dtypes

import jax
import jax.numpy as jnp

import concourse.bass as bass
import concourse.mybir as mybir
import concourse.tile as tile
from concourse import bacc
from concourse.bass_utils import run_bass_kernel_spmd
from concourse.tile_rust import add_dep_helper

BF16 = ml_dtypes.bfloat16
H = 1024
I = 2816
E = 9
TOPK = 2
NCORES = 8
P = 128
HK = H // P       # 8 partition-tiles over H
IK = I // P       # 22 partition-tiles over I
NT = 512          # token tile (PSUM bank = 512 fp32)

LAST_EXEC_NS = None          # set when BASS_TRACE=1 (read by test harness)
_PROGRAM_CACHE = {}


def _route(x, Wr):
    """Router on jax-CPU, eager, with the reference's exact op sequence."""
    cpu = jax.devices("cpu")[0]
    with jax.default_device(cpu):
        xj = jnp.asarray(np.asarray(x))
        wj = jnp.asarray(np.asarray(Wr))
        logits = jnp.einsum("bsh,he->bse", xj, wj)
        probs = jax.nn.softmax(logits, axis=-1)
        topk_w, topk_idx = jax.lax.top_k(probs, TOPK)
        topk_w = topk_w / jnp.sum(topk_w, axis=-1, keepdims=True)
    T = x.shape[0] * x.shape[1]
    return (np.asarray(topk_idx).reshape(T, TOPK),
            np.asarray(topk_w).astype(np.float32).reshape(T, TOPK))


def _token_units(CA, CB):
    """(slot, col0, ncols, localcol0) units covering [0, CA+CB)."""
    units = []
    for c0 in range(0, CA, NT):
        units.append((0, c0, min(NT, CA - c0), c0))
    for c0 in range(0, CB, NT):
        units.append((1, CA + c0, min(NT, CB - c0), c0))
    return units


def _build_program(CA, CB):
    C = CA + CB
    nc = bacc.Bacc("TRN2", target_bir_lowering=False, debug=False,
                   num_devices=NCORES)
    bf = mybir.dt.bfloat16
    f32 = mybir.dt.float32
    xt_d = nc.dram_tensor("xt", [HK, P, C], bf, kind="ExternalInput")
    wg_d = nc.dram_tensor("wg", [2, IK, P, HK, P], bf, kind="ExternalInput")
    wu_d = nc.dram_tensor("wu", [2, IK, P, HK, P], bf, kind="ExternalInput")
    wd_d = nc.dram_tensor("wd", [2, HK, P, IK, P], bf, kind="ExternalInput")
    y_d = nc.dram_tensor("y", [HK, P, C], bf, kind="ExternalOutput")

    units = _token_units(CA, CB)

    with tile.TileContext(nc) as tc:
        with (
            tc.tile_pool(name="xpool", bufs=1) as xpool,
            tc.tile_pool(name="hpool", bufs=1) as hpool,
            tc.tile_pool(name="wpool", bufs=2) as wpool,
            tc.tile_pool(name="wdpool", bufs=2) as wdpool,
            tc.tile_pool(name="gpool", bufs=3) as gpool,
            tc.tile_pool(name="ypool", bufs=3) as ypool,
            tc.tile_pool(name="ps1", bufs=3, space="PSUM") as ps1,
            tc.tile_pool(name="ps2", bufs=2, space="PSUM") as ps2,
        ):
            # Warmup: throwaway matmuls on a zeroed tile keep the PE busy
            # through the startup DMA window so the HAM clock gate reaches
            # 8/8 before the real matmul stream begins. They run in the ps2
            # ring's first rotation slots, costing no extra PSUM bank.
            with tc.high_priority():
                wmt = gpool.tile([P, NT], bf, tag="warm", name="warm", bufs=1)
                nc.vector.memset(wmt[:], 0)
                for r in range(5):
                    pw = ps2.tile([P, NT], f32, tag="pd", name=f"pw{r}")
                    nc.tensor.matmul(pw, wmt[:, 0:P], wmt[:],
                                     start=True, stop=True)
            # resident tokens: one tile per H k-tile so the k-th matmul of
            # the first accumulation group only waits on its own DMA
            xts = []
            with tc.high_priority():
                for k in range(HK):
                    xk = xpool.tile([P, C], bf, tag=f"xt{k}", name=f"xt{k}")
                    nc.sync.dma_start(xk[:], xt_d[k])
                    xts.append(xk)
            hid = [hpool.tile([P, IK, CA], bf, tag="hidA", name="hidA"),
                   hpool.tile([P, IK, CB], bf, tag="hidB", name="hidB")]

            # phase 1: gate/up + silu*up, streaming Wg/Wu by I-tile
            p1_marker = None
            for i in range(IK):
                wgt, wut = [], []
                for s in (0, 1):
                    # The i=0 slot-0 weights gate the first real matmuls:
                    # issue them on the otherwise-idle GPSIMD SWDGE ring so
                    # they do not queue (or share issue slots) behind the
                    # x-token loads on the sync HWDGE ring.
                    first = i == 0 and s == 0
                    eng = nc.gpsimd if first else nc.sync
                    g = wpool.tile([P, HK, P], bf, tag=f"wg{s}", name=f"wg{s}")
                    dg = eng.dma_start(g[:], wg_d[s, i])
                    u = wpool.tile([P, HK, P], bf, tag=f"wu{s}", name=f"wu{s}")
                    du = eng.dma_start(u[:], wu_d[s, i])
                    wgt.append(g)
                    wut.append(u)
                    if first:
                        # keep the first-needed weight loads ahead of prefetch
                        dg.ins.bass_priority = 0
                        du.ins.bass_priority = 0
                for (s, c0, n, lc) in units:
                    pg = ps1.tile([P, NT], f32, tag="pg", name="pg")[:, :n]
                    pu = ps1.tile([P, NT], f32, tag="pu", name="pu")[:, :n]
                    for k in range(HK):
                        mm = nc.tensor.matmul(pg, wgt[s][:, k, :],
                                              xts[k][:, c0:c0 + n],
                                              start=(k == 0), stop=(k == HK - 1))
                        if i == 2 and p1_marker is None:
                            p1_marker = mm
                    for k in range(HK):
                        nc.tensor.matmul(pu, wut[s][:, k, :],
                                         xts[k][:, c0:c0 + n],
                                         start=(k == 0), stop=(k == HK - 1))
                    gt = gpool.tile([P, NT], bf, tag="gt", name="gt")[:, :n]
                    nc.scalar.activation(gt, pg,
                                         mybir.ActivationFunctionType.Silu)
                    nc.vector.tensor_mul(hid[s][:, i, lc:lc + n], gt, pu)

            # phase 2: down proj, streaming Wd by H-tile
            for j in range(HK):
                wdt = []
                for s in (0, 1):
                    d = wdpool.tile([P, IK, P], bf, tag=f"wd{s}", name=f"wd{s}")
                    dd = nc.sync.dma_start(d[:], wd_d[s, j])
                    if j < 2 and p1_marker is not None:
                        # keep the big Wd prefetches out of the startup
                        # window where they compete with first-needed DMAs
                        add_dep_helper(p1_marker.ins, dd.ins, sync=False,
                                       reason="delay wd prefetch")
                    wdt.append(d)
                for (s, c0, n, lc) in units:
                    pd = ps2.tile([P, NT], f32, tag="pd", name="pd")[:, :n]
                    for i in range(IK):
                        nc.tensor.matmul(pd, wdt[s][:, i, :],
                                         hid[s][:, i, lc:lc + n],
                                         start=(i == 0), stop=(i == IK - 1))
                    yt = ypool.tile([P, NT], bf, tag="yt", name="yt")[:, :n]
                    nc.vector.tensor_copy(yt, pd)
                    nc.sync.dma_start(y_d[j, :, c0:c0 + n], yt)

    nc.compile()
    return nc


def _pack_gateup(w):        # [H, I] -> [IK, P(ki), HK, P(ii)] contiguous
    return np.ascontiguousarray(
        w.reshape(HK, P, IK, P).transpose(2, 1, 0, 3))


def _pack_down(w):          # [I, H] -> [HK, P(ii), IK, P(jj)] contiguous
    return np.ascontiguousarray(
        w.reshape(IK, P, HK, P).transpose(2, 1, 0, 3))


def kernel(x, Wr, Wg, Wu, Wd):
    global LAST_EXEC_NS
    x = np.asarray(x)
    B, S, _ = x.shape
    T = B * S
    xf = np.asarray(x, dtype=np.float32).reshape(T, H)

    idx, w = _route(x, Wr)

    # per-expert token lists and combine weights
    toks, cws = [], []
    for e in range(E):
        m = idx == e
        te = np.nonzero(m.any(axis=1))[0]
        toks.append(te)
        cws.append((w * m).sum(axis=1)[te].astype(np.float32))
    counts = np.array([len(t) for t in toks])

    s_star = int(np.argmax(counts))           # split expert (slot B)
    owners = [e for e in range(E) if e != s_star]   # slot A expert per core
    CA = max(2, int(counts[owners].max()))
    CB = max(2, int(-(-counts[s_star] // NCORES)))
    C = CA + CB

    key = (CA, CB)
    if key not in _PROGRAM_CACHE:
        _PROGRAM_CACHE[key] = _build_program(CA, CB)
    nc = _PROGRAM_CACHE[key]

    Wgb = np.asarray(Wg, dtype=BF16)
    Wub = np.asarray(Wu, dtype=BF16)
    Wdb = np.asarray(Wd, dtype=BF16)
    wg_s = _pack_gateup(Wgb[s_star])
    wu_s = _pack_gateup(Wub[s_star])
    wd_s = _pack_down(Wdb[s_star])

    tb = toks[s_star]
    in_maps = []
    for c in range(NCORES):
        ea = owners[c]
        ta = toks[ea]
        tbc = tb[c * CB:(c + 1) * CB]
        xt = np.zeros((H, C), dtype=BF16)
        xt[:, :len(ta)] = xf[ta].T
        xt[:, CA:CA + len(tbc)] = xf[tbc].T
        in_maps.append({
            "xt": np.ascontiguousarray(xt.reshape(HK, P, C)),
            "wg": np.stack([_pack_gateup(Wgb[ea]), wg_s]),
            "wu": np.stack([_pack_gateup(Wub[ea]), wu_s]),
            "wd": np.stack([_pack_down(Wdb[ea]), wd_s]),
        })

    res = run_bass_kernel_spmd(nc, in_maps, core_ids=list(range(NCORES)))
    LAST_EXEC_NS = res.exec_time_ns

    out = np.zeros((T, H), dtype=np.float32)
    for c in range(NCORES):
        y = np.asarray(res.results[c]["y"]).astype(np.float32)
        y = y.reshape(H, C).T                 # [C, H]
        ea = owners[c]
        ta = toks[ea]
        if len(ta):
            out[ta] += y[:len(ta)] * cws[ea][:, None]
        tbc = tb[c * CB:(c + 1) * CB]
        if len(tbc):
            wb = cws[s_star][c * CB:(c + 1) * CB]
            out[tbc] += y[CA:CA + len(tbc)] * wb[:, None]

    return out.reshape(B, S, H)



# revision 19
# speedup vs baseline: 1.0055x; 1.0055x over previous
"""MoE MLP (9 experts, top-2 routing) on 8 TRN2 NeuronCores.

Strategy: expert-parallel. The router (tiny) runs on host CPU with the exact
jax ops of the reference so top-2 selection matches bitwise. Tokens are
gathered per expert on host; the largest expert is split across all 8 cores
(slot B), each core additionally owns one of the remaining 8 experts
(slot A). Every core runs the same SPMD Bass program (shapes baked from the
actual routing at call time): gate/up matmuls (bf16, fp32 PSUM), silu*up,
down matmul, all with features on partitions and tokens on the free dim so
no transposes are needed. Host applies combine weights and scatter-adds.

The matmul stream is PE-streaming-bound (~220ns/token column), so the
remaining edges are optimized: warmup matmuls on a zeroed tile keep the PE
busy (and the HAM clock-gate warming) through the startup DMA window, the
first-needed weight loads carry raised DMA priority, and the output is
returned as bf16 to halve the output drain.
"""

import os

# The tunneled NeuronCores can be left wedged (NRT_EXEC_UNIT_UNRECOVERABLE)
# by a previous process; resetting cores at NRT init makes runs reliable.
os.environ.setdefault("NEURON_RT_RESET_CORES", "1")

import numpy as np
import ml_dtypes

import jax
import jax.numpy as jnp

import concourse.bass as bass
import concourse.mybir as mybir
import concourse.tile as tile
from concourse import bacc
from concourse.bass_utils import run_bass_kernel_spmd
from concourse.tile_rust import add_dep_helper

BF16 = ml_dtypes.bfloat16
H = 1024
I = 2816
E = 9
TOPK = 2
NCORES = 8
P = 128
HK = H // P       # 8 partition-tiles over H
IK = I // P       # 22 partition-tiles over I
NT = 512          # token tile (PSUM bank = 512 fp32)

LAST_EXEC_NS = None          # set when BASS_TRACE=1 (read by test harness)
_PROGRAM_CACHE = {}


def _route(x, Wr):
    """Router on jax-CPU, eager, with the reference's exact op sequence."""
    cpu = jax.devices("cpu")[0]
    with jax.default_device(cpu):
        xj = jnp.asarray(np.asarray(x))
        wj = jnp.asarray(np.asarray(Wr))
        logits = jnp.einsum("bsh,he->bse", xj, wj)
        probs = jax.nn.softmax(logits, axis=-1)
        topk_w, topk_idx = jax.lax.top_k(probs, TOPK)
        topk_w = topk_w / jnp.sum(topk_w, axis=-1, keepdims=True)
    T = x.shape[0] * x.shape[1]
    return (np.asarray(topk_idx).reshape(T, TOPK),
            np.asarray(topk_w).astype(np.float32).reshape(T, TOPK))


def _token_units(CA, CB):
    """(slot, col0, ncols, localcol0) units covering [0, CA+CB)."""
    units = []
    for c0 in range(0, CA, NT):
        units.append((0, c0, min(NT, CA - c0), c0))
    for c0 in range(0, CB, NT):
        units.append((1, CA + c0, min(NT, CB - c0), c0))
    return units


def _build_program(CA, CB):
    C = CA + CB
    nc = bacc.Bacc("TRN2", target_bir_lowering=False, debug=False,
                   num_devices=NCORES)
    bf = mybir.dt.bfloat16
    f32 = mybir.dt.float32
    xt_d = nc.dram_tensor("xt", [HK, P, C], bf, kind="ExternalInput")
    wg_d = nc.dram_tensor("wg", [2, IK, P, HK, P], bf, kind="ExternalInput")
    wu_d = nc.dram_tensor("wu", [2, IK, P, HK, P], bf, kind="ExternalInput")
    wd_d = nc.dram_tensor("wd", [2, HK, P, IK, P], bf, kind="ExternalInput")
    y_d = nc.dram_tensor("y", [HK, P, C], bf, kind="ExternalOutput")

    units = _token_units(CA, CB)

    with tile.TileContext(nc) as tc:
        with (
            tc.tile_pool(name="xpool", bufs=1) as xpool,
            tc.tile_pool(name="hpool", bufs=1) as hpool,
            tc.tile_pool(name="wpool", bufs=2) as wpool,
            tc.tile_pool(name="wdpool", bufs=2) as wdpool,
            tc.tile_pool(name="gpool", bufs=3) as gpool,
            tc.tile_pool(name="ypool", bufs=3) as ypool,
            tc.tile_pool(name="ps1", bufs=3, space="PSUM") as ps1,
            tc.tile_pool(name="ps2", bufs=2, space="PSUM") as ps2,
        ):
            # Warmup: throwaway matmuls on a zeroed tile keep the PE busy
            # through the startup DMA window so the HAM clock gate reaches
            # 8/8 before the real matmul stream begins. They run in the ps2
            # ring's first rotation slots, costing no extra PSUM bank.
            with tc.high_priority():
                wmt = gpool.tile([P, NT], bf, tag="warm", name="warm", bufs=1)
                nc.vector.memset(wmt[:], 0)
                for r in range(6):
                    pw = ps2.tile([P, NT], f32, tag="pd", name=f"pw{r}")
                    nc.tensor.matmul(pw, wmt[:, 0:P], wmt[:],
                                     start=True, stop=True)
            # resident tokens: one tile per H k-tile so the k-th matmul of
            # the first accumulation group only waits on its own DMA
            xts = []
            with tc.high_priority():
                for k in range(HK):
                    xk = xpool.tile([P, C], bf, tag=f"xt{k}", name=f"xt{k}")
                    nc.sync.dma_start(xk[:], xt_d[k])
                    xts.append(xk)
            hid = [hpool.tile([P, IK, CA], bf, tag="hidA", name="hidA"),
                   hpool.tile([P, IK, CB], bf, tag="hidB", name="hidB")]

            # phase 1: gate/up + silu*up, streaming Wg/Wu by I-tile
            p1_marker = None
            for i in range(IK):
                wgt, wut = [], []
                for s in (0, 1):
                    g = wpool.tile([P, HK, P], bf, tag=f"wg{s}", name=f"wg{s}")
                    dg = nc.sync.dma_start(g[:], wg_d[s, i])
                    u = wpool.tile([P, HK, P], bf, tag=f"wu{s}", name=f"wu{s}")
                    du = nc.sync.dma_start(u[:], wu_d[s, i])
                    wgt.append(g)
                    wut.append(u)
                    if i == 0 and s == 0:
                        # keep the first-needed weight loads ahead of prefetch
                        dg.ins.bass_priority = 0
                        du.ins.bass_priority = 0
                for (s, c0, n, lc) in units:
                    pg = ps1.tile([P, NT], f32, tag="pg", name="pg")[:, :n]
                    pu = ps1.tile([P, NT], f32, tag="pu", name="pu")[:, :n]
                    for k in range(HK):
                        mm = nc.tensor.matmul(pg, wgt[s][:, k, :],
                                              xts[k][:, c0:c0 + n],
                                              start=(k == 0), stop=(k == HK - 1))
                        if i == 2 and p1_marker is None:
                            p1_marker = mm
                    for k in range(HK):
                        nc.tensor.matmul(pu, wut[s][:, k, :],
                                         xts[k][:, c0:c0 + n],
                                         start=(k == 0), stop=(k == HK - 1))
                    gt = gpool.tile([P, NT], bf, tag="gt", name="gt")[:, :n]
                    nc.scalar.activation(gt, pg,
                                         mybir.ActivationFunctionType.Silu)
                    nc.vector.tensor_mul(hid[s][:, i, lc:lc + n], gt, pu)

            # phase 2: down proj, streaming Wd by H-tile
            for j in range(HK):
                wdt = []
                for s in (0, 1):
                    d = wdpool.tile([P, IK, P], bf, tag=f"wd{s}", name=f"wd{s}")
                    dd = nc.sync.dma_start(d[:], wd_d[s, j])
                    if j < 2 and p1_marker is not None:
                        # keep the big Wd prefetches out of the startup
                        # window where they compete with first-needed DMAs
                        add_dep_helper(p1_marker.ins, dd.ins, sync=False,
                                       reason="delay wd prefetch")
                    wdt.append(d)
                for (s, c0, n, lc) in units:
                    pd = ps2.tile([P, NT], f32, tag="pd", name="pd")[:, :n]
                    for i in range(IK):
                        nc.tensor.matmul(pd, wdt[s][:, i, :],
                                         hid[s][:, i, lc:lc + n],
                                         start=(i == 0), stop=(i == IK - 1))
                    yt = ypool.tile([P, NT], bf, tag="yt", name="yt")[:, :n]
                    nc.vector.tensor_copy(yt, pd)
                    nc.sync.dma_start(y_d[j, :, c0:c0 + n], yt)

    nc.compile()
    return nc


def _pack_gateup(w):        # [H, I] -> [IK, P(ki), HK, P(ii)] contiguous
    return np.ascontiguousarray(
        w.reshape(HK, P, IK, P).transpose(2, 1, 0, 3))


def _pack_down(w):          # [I, H] -> [HK, P(ii), IK, P(jj)] contiguous
    return np.ascontiguousarray(
        w.reshape(IK, P, HK, P).transpose(2, 1, 0, 3))


def kernel(x, Wr, Wg, Wu, Wd):
    global LAST_EXEC_NS
    x = np.asarray(x)
    B, S, _ = x.shape
    T = B * S
    xf = np.asarray(x, dtype=np.float32).reshape(T, H)

    idx, w = _route(x, Wr)

    # per-expert token lists and combine weights
    toks, cws = [], []
    for e in range(E):
        m = idx == e
        te = np.nonzero(m.any(axis=1))[0]
        toks.append(te)
        cws.append((w * m).sum(axis=1)[te].astype(np.float32))
    counts = np.array([len(t) for t in toks])

    s_star = int(np.argmax(counts))           # split expert (slot B)
    owners = [e for e in range(E) if e != s_star]   # slot A expert per core
    CA = max(2, int(counts[owners].max()))
    CB = max(2, int(-(-counts[s_star] // NCORES)))
    C = CA + CB

    key = (CA, CB)
    if key not in _PROGRAM_CACHE:
        _PROGRAM_CACHE[key] = _build_program(CA, CB)
    nc = _PROGRAM_CACHE[key]

    Wgb = np.asarray(Wg, dtype=BF16)
    Wub = np.asarray(Wu, dtype=BF16)
    Wdb = np.asarray(Wd, dtype=BF16)
    wg_s = _pack_gateup(Wgb[s_star])
    wu_s = _pack_gateup(Wub[s_star])
    wd_s = _pack_down(Wdb[s_star])

    tb = toks[s_star]
    in_maps = []
    for c in range(NCORES):
        ea = owners[c]
        ta = toks[ea]
        tbc = tb[c * CB:(c + 1) * CB]
        xt = np.zeros((H, C), dtype=BF16)
        xt[:, :len(ta)] = xf[ta].T
        xt[:, CA:CA + len(tbc)] = xf[tbc].T
        in_maps.append({
            "xt": np.ascontiguousarray(xt.reshape(HK, P, C)),
            "wg": np.stack([_pack_gateup(Wgb[ea]), wg_s]),
            "wu": np.stack([_pack_gateup(Wub[ea]), wu_s]),
            "wd": np.stack([_pack_down(Wdb[ea]), wd_s]),
        })

    res = run_bass_kernel_spmd(nc, in_maps, core_ids=list(range(NCORES)))
    LAST_EXEC_NS = res.exec_time_ns

    out = np.zeros((T, H), dtype=np.float32)
    for c in range(NCORES):
        y = np.asarray(res.results[c]["y"]).astype(np.float32)
        y = y.reshape(H, C).T                 # [C, H]
        ea = owners[c]
        ta = toks[ea]
        if len(ta):
            out[ta] += y[:len(ta)] * cws[ea][:, None]
        tbc = tb[c * CB:(c + 1) * CB]
        if len(tbc):
            wb = cws[s_star][c * CB:(c + 1) * CB]
            out[tbc] += y[CA:CA + len(tbc)] * wb[:, None]

    return out.reshape(B, S, H)

